# revision 1
# baseline (speedup 1.0000x reference)
"""Trainium2 Bass kernel for nn_Attention (buggy-reshape attention), 8-core SPMD.

Math (reference): q/k/v = (x @ W).reshape entangles batch and head. Each of the
256 (h,b) "chunks" is a contiguous 64-row block of the projected (16384, 512)
matrices:
  K_c = XK[64c:64c+64, :]            (64=A, 512=M)  -- used as-is
  Q_c = XQ[64c:64c+64, :].reshape(512, 64)
  V_c = XV[64c:64c+64, :].reshape(512, 64)
  out_c = softmax(Q_c @ K_c, -1) @ V_c ; final[b] = relu(mean_h out_(h,b) + x_b @ Wr)

Chunk (h,b) touches only x[4h + b//8, 64*(b%8):64*(b%8)+64, :]. We shard by
OUTPUT batch: core d owns batches 4d..4d+3 (all 8 heads) and is handed exactly
the x rows it needs -> zero collectives; head-mean is local.

Per-core layouts (m-permutation p = 64*s + r where m = 8*r + s; same perm used
for the n axis via host-permuted Wk columns):
  S^T tiles (n'-part, p-free) = Ksb_slice.T @ QTall_slice ; softmax over n'
  (partition axis; no max subtraction -- scores are O(+-50), exp fits fp32
  easily); column sums via ones-matmul; O^T = V_perm.T @ expS, normalized by
  approx-reciprocal broadcast; 1/8 head-mean folded into Wv.

Schedule: software-pipelined over the 16 chunk-pairs. Iteration (g,c) runs
S-matmuls of pair (g,c) split around the projections of group g+1 and the
sum/O/normalize of pair (g,c-1), keeping PE/Act/DVE all near-continuously
busy.  Exp is the metronome: 64 x [128,1024] activations on the scalar
engine.  All psum->sbuf drains live on DVE (gpsimd cannot touch psum);
gpsimd only dispatches gated bulk DMAs.

dtypes: everything fp16 into the PE except the exp output (bf16, needs range)
and so the V/ones stationaries of the O/sum matmuls are bf16 to match es;
psum always fp32.  M=64 matmuls are column-paired onto disjoint PE halves via
psum base-partition-64 outputs (auto tile_position); two accumulation chains
never interleave within the same psum (bank, partition-half).
"""

import os
import sys

import numpy as np

sys.path.insert(0, "/opt/trn_rl_repo")

import concourse.bass as bass
import concourse.bacc as bacc
import concourse.mybir as mybir
from concourse.tile import TileContext

FP = mybir.dt.float32
BF = mybir.dt.bfloat16
F16 = mybir.dt.float16
AF = mybir.ActivationFunctionType
ALU = mybir.AluOpType

B, M, E, H, A = 32, 512, 256, 8, 64
NCORES = 8

# m (and n) permutation: p = 64*s + r  <->  m = 8*r + s
_M_OF_P = np.array([8 * (p % 64) + p // 64 for p in range(512)])
_P_OF_M = np.array([64 * (m % 8) + m // 8 for m in range(512)])


def build_core_graph():
    nc = bacc.Bacc(target_bir_lowering=False)

    xaT_e = nc.declare_dram_parameter("xaT", [E, 2048], F16, isOutput=False)
    xoT_e = nc.declare_dram_parameter("xoT", [E, 2048], F16, isOutput=False)
    wqk_e = nc.declare_dram_parameter("wqk", [E, 1024], F16, isOutput=False)
    wvr_e = nc.declare_dram_parameter("wvr", [E, 512 + A], F16, isOutput=False)
    out_e = nc.declare_dram_parameter("out", [A, 2048], FP, isOutput=True)

    with TileContext(nc) as tc:
        from contextlib import ExitStack

        with ExitStack() as ctx:
            const = ctx.enter_context(tc.tile_pool(name="const", bufs=1))
            qt_pool = ctx.enter_context(tc.tile_pool(name="qt", bufs=2))
            ksb_pool = ctx.enter_context(tc.tile_pool(name="ksb", bufs=8))
            vsb_pool = ctx.enter_context(tc.tile_pool(name="vsb", bufs=16))
            exps_pool = ctx.enter_context(tc.tile_pool(name="exps", bufs=4))
            esum_pool = ctx.enter_context(tc.tile_pool(name="esum", bufs=2))
            misc_pool = ctx.enter_context(tc.tile_pool(name="misc", bufs=4))
            acc_pool = ctx.enter_context(tc.tile_pool(name="acc", bufs=2))

            pp_psum = ctx.enter_context(tc.tile_pool(name="pp", bufs=2, space="PSUM"))
            st_psum = ctx.enter_context(tc.tile_pool(name="st", bufs=2, space="PSUM"))
            so_psum = ctx.enter_context(tc.tile_pool(name="so", bufs=2, space="PSUM"))

            # ---- input tiles ----
            xaT_t = const.tile([128, 2, 4, 4, 2, A], F16, tag="xaT")
            xoT_t = const.tile([128, 2, 2048], F16, tag="xoT")
            wqk = const.tile([128, 2, 1024], F16, tag="wqk")
            wvr = const.tile([128, 2, 512 + A], F16, tag="wvr")
            ones = const.tile([128, A], BF, tag="ones")
            nc.vector.memset(ones[:], 1.0)

            # startup-critical loads: K(0,0) fires first, so wkp + xaT_g0
            # lead; wq/wvr next; bulk (xaT g1:3, xoT) is gated behind group-0
            # compute below so it cannot steal DMA engines from these.
            nc.sync.dma_start(
                out=xaT_t[:, :, 0, :, :, :],
                in_=xaT_e[:, 0:512].rearrange("(k p) (c t r) -> p k c t r",
                                              k=2, c=4, t=2))
            nc.scalar.dma_start(
                out=wqk[:, :, 512:1024],
                in_=wqk_e[:, 512:1024].rearrange("(k p) n -> p k n", k=2))
            nc.sync.dma_start(
                out=wqk[:, :, 0:512],
                in_=wqk_e[:, 0:512].rearrange("(k p) n -> p k n", k=2))
            nc.scalar.dma_start(
                out=wvr[:, :, :],
                in_=wvr_e[:, :].rearrange("(k p) n -> p k n", k=2))

            wq = [wqk[:, k, 0:512] for k in range(2)]
            wkp = [wqk[:, k, 512:1024] for k in range(2)]
            wv8 = [wvr[:, k, 0:512] for k in range(2)]
            wv8v = [wv8[k].rearrange("p (hh sp a) -> p hh sp a", hh=4, sp=2)
                    for k in range(2)]
            wr = [wvr[:, k, 512:512 + A] for k in range(2)]
            xoT = [xoT_t[:, k, :] for k in range(2)]

            acc2 = [acc_pool.tile([128, 512], FP, tag="acc", name="acc")
                    for _ in range(2)]
            prt2 = const.tile([128, 2, 512], FP, tag="prt2")

            qtall_t = {}
            ksb_t = {}
            vsb_t = {}
            es_t = {}
            esum_t = {}

            # ---------------- building blocks ----------------
            def q_proj2(g, sp):
                # two s-blocks per psum bank: the two accumulation chains are
                # strictly sequential (closed-chain data in a bank is safe, as
                # the V projection's chain pattern validates), and one copy
                # drains both -> half the psum handoffs.
                if sp == 0:
                    qtall_t[g] = qt_pool.tile([128, 8, 4, A], F16, tag="qt",
                                              name="qtall")
                qp4 = pp_psum.tile([128, 2, 4, A], FP, tag="pp", name="qp4")
                for sh in range(2):
                    s = 2 * sp + sh
                    for k in range(2):
                        for par in range(2):
                            nc.tensor.matmul(
                                qp4[64 * par:64 * par + 64, sh, :, :],
                                wq[k][:, 64 * s:64 * s + 64],
                                xaT_t[:, k, g, :, par, :],
                                start=(k == 0), stop=(k == 1),
                                skip_group_check=True)
                if sp % 2 == 0:
                    nc.scalar.copy(qtall_t[g][:, 2 * sp:2 * sp + 2, :, :],
                                   qp4[:, :, :, :])
                else:
                    nc.vector.tensor_copy(
                        qtall_t[g][:, 2 * sp:2 * sp + 2, :, :],
                        qp4[:, :, :, :])

            def k_proj(g, c):
                kp2 = pp_psum.tile([128, 512], FP, tag="pp", name="kp2")
                for k in range(2):
                    for par in range(2):
                        nc.tensor.matmul(kp2[64 * par:64 * par + 64, :],
                                         xaT_t[:, k, g, c, par, :], wkp[k],
                                         start=(k == 0), stop=(k == 1),
                                         skip_group_check=True)
                ksb_t[(g, c)] = ksb_pool.tile([128, 512], F16, tag="ksb",
                                              name="ksb")
                nc.scalar.copy(ksb_t[(g, c)][:, 0:256], kp2[:, 0:256])
                nc.vector.tensor_copy(ksb_t[(g, c)][:, 256:512],
                                      kp2[:, 256:512])

            def v_proj(g, c):
                # psum lands directly in V_perm layout (sp -> partition half,
                # par -> free dim); chains (0,0)/(1,1) complete before
                # (0,1)/(1,0) so no psum partition-half hosts two chains.
                pv4 = pp_psum.tile([128, 4, 2, A], FP, tag="pp", name="pv4")
                for grp in (((0, 0), (1, 1)), ((0, 1), (1, 0))):
                    for k in range(2):
                        for par, sp in grp:
                            nc.tensor.matmul(
                                pv4[64 * sp:64 * sp + 64, :, par, :],
                                xaT_t[:, k, g, c, par, :],
                                wv8v[k][:, :, sp, :],
                                start=(k == 0), stop=(k == 1),
                                skip_group_check=True)
                vsb_e = vsb_pool.tile([128, 4, A], BF, tag="vsb", name="vsbe")
                vsb_o = vsb_pool.tile([128, 4, A], BF, tag="vsb", name="vsbo")
                nc.vector.tensor_copy(vsb_e[:, :, :], pv4[:, :, 0, :])
                nc.vector.tensor_copy(vsb_o[:, :, :], pv4[:, :, 1, :])
                vsb_t[(g, c)] = (vsb_e, vsb_o)

            def s_units(g, c, par):
                # S^T matmuls + exp for one parity (2 units: half 0/1).
                # Unpaired on the PE row halves on purpose: pairing trips the
                # chip power governor (~20% global clock drop, measured).
                if par == 0:
                    es_t[(g, c)] = {}
                es = exps_pool.tile([128, 4, 512], BF, tag="exps", name="es")
                es_t[(g, c)][par] = es
                ksb = ksb_t[(g, c)]
                qtall = qtall_t[g]
                for half in range(2):
                    st = st_psum.tile([128, 2, 512], FP, tag="st", name="st")
                    for q2 in range(2):
                        kn = 2 * half + q2
                        nc.tensor.matmul(
                            st[:, q2, :],
                            ksb[64 * par:64 * par + 64,
                                128 * kn:128 * kn + 128],
                            qtall[64 * par:64 * par + 64, :, c, :],
                            start=True, stop=True)
                    nc.scalar.activation(es[:, 2 * half:2 * half + 2, :],
                                         st[:], AF.Exp)

            def presum(g, c):
                # pre-sum kn pairs on DVE (bf16, 2x mode) so the ones-matmul
                # streams half the columns; emitted at the iteration top so it
                # overlaps S-matmuls instead of stalling the colsum.
                es = es_t[(g, c)]
                esum = {}
                for par in range(2):
                    esum[par] = esum_pool.tile([128, 2, 512], BF, tag="esum",
                                               name="esum")
                    nc.vector.tensor_add(esum[par][:], es[par][:, 0:2, :],
                                         es[par][:, 2:4, :])
                esum_t[(g, c)] = esum

            def sum_o(g, c, fine=False):
                # column sums + O^T, then normalize and accumulate
                es = es_t.pop((g, c))
                vsb = vsb_t.pop((g, c))
                q = c % 2
                first = (g == 0 and c < 2)
                esum = esum_t.pop((g, c))
                sumb2 = so_psum.tile([128, 512], FP, tag="so", name="sumb2")
                for kn2 in range(2):
                    for par in range(2):
                        nc.tensor.matmul(sumb2[64 * par:64 * par + 64, :],
                                         ones[:, 0:A], esum[par][:, kn2, :],
                                         start=(kn2 == 0), stop=(kn2 == 1),
                                         skip_group_check=True)
                ot2 = so_psum.tile([128, 512], FP, tag="so", name="ot2")
                for kn in range(4):
                    for par in range(2):
                        nc.tensor.matmul(ot2[64 * par:64 * par + 64, :],
                                         vsb[par][:, kn, :], es[par][:, kn, :],
                                         start=(kn == 0), stop=(kn == 3),
                                         skip_group_check=True)
                recipb2 = misc_pool.tile([128, 512], FP, tag="recip",
                                         name="recipb2")
                halves = ((0, 512),) if not fine else ((0, 256), (256, 512))
                for lo, hi in halves:
                    nc.vector.reciprocal_approx_fast(out=recipb2[:, lo:hi],
                                                     in_=sumb2[:, lo:hi])
                    if first:
                        nc.vector.tensor_mul(acc2[q][:, lo:hi], ot2[:, lo:hi],
                                             recipb2[:, lo:hi])
                    else:
                        otmp2 = misc_pool.tile([128, 512], FP, tag="otmp",
                                               name="otmp2")
                        nc.vector.tensor_mul(otmp2[:, lo:hi], ot2[:, lo:hi],
                                             recipb2[:, lo:hi])
                        nc.vector.tensor_add(acc2[q][:, lo:hi],
                                             acc2[q][:, lo:hi],
                                             otmp2[:, lo:hi])

            def wr_proj(q):
                rp2 = pp_psum.tile([128, 512], FP, tag="pp", name="rp2")
                for k in range(2):
                    for par in range(2):
                        nc.tensor.matmul(
                            rp2[64 * par:64 * par + 64, :],
                            wr[k],
                            xoT[k][:, 512 * (2 * q + par):
                                   512 * (2 * q + par) + 512],
                            start=(k == 0), stop=(k == 1),
                            skip_group_check=True)
                nc.scalar.copy(prt2[:, q, :], rp2[:])

            def epilogue(q, fine=False):
                pre2 = misc_pool.tile([128, 512], FP, tag="pre", name="pre2")
                outsb2 = misc_pool.tile([128, 512], FP, tag="outsb",
                                        name="outsb2")
                halves = ((0, 512),) if not fine else ((0, 256), (256, 512))
                for lo, hi in halves:
                    nc.vector.tensor_add(pre2[:, lo:hi], acc2[q][:, lo:hi],
                                         prt2[:, q, lo:hi])
                    nc.vector.tensor_scalar_max(outsb2[:, lo:hi],
                                                pre2[:, lo:hi], 0.0)
                    for par in range(2):
                        nc.sync.dma_start(
                            out=out_e[:, 512 * (2 * q + par) + lo:
                                      512 * (2 * q + par) + hi],
                            in_=outsb2[64 * par:64 * par + 64, lo:hi])

            # ---------------- prologue: group-0 projections ----------------
            k_proj(0, 0)
            for sp in range(4):
                q_proj2(0, sp)
                if sp == 0:
                    # Bulk loads gated behind early group-0 compute via junk
                    # WAW stores (overwritten by the DMAs) so their transfers
                    # don't steal DMA engines from the startup-critical ones.
                    nc.gpsimd.tensor_copy(xaT_t[0:1, 0, 1, 0, 0, 0:4],
                                          qtall_t[0][0:1, 0, 0, 0:4])
                    nc.gpsimd.tensor_copy(xoT_t[0:1, 0, 0:4],
                                          qtall_t[0][0:1, 0, 0, 0:4])
                    nc.gpsimd.dma_start(
                        out=xaT_t[:, :, 1:4, :, :, :],
                        in_=xaT_e[:, 512:2048].rearrange(
                            "(k p) (g c t r) -> p k g c t r", k=2, g=3, c=4,
                            t=2))
                    nc.gpsimd.dma_start(
                        out=xoT_t[:, :, :],
                        in_=xoT_e[:, :].rearrange("(k p) n -> p k n", k=2))
            # first S-units fire before the remaining group-0 projections
            # so the exp metronome starts as early as possible
            s_units(0, 0, par=0)
            v_proj(0, 0)
            for c in range(1, 4):
                k_proj(0, c)
                v_proj(0, c)

            # ---------------- steady loop over the 16 pairs ----------------
            pairs = [(g, c) for g in range(4) for c in range(4)]
            for i, (g, c) in enumerate(pairs):
                if i > 0:
                    presum(*pairs[i - 1])
                    s_units(g, c, par=0)
                else:
                    s_units(g, c, par=1)
                if g < 3:
                    k_proj(g + 1, c)
                    v_proj(g + 1, c)
                if i > 0:
                    sum_o(*pairs[i - 1])
                    s_units(g, c, par=1)
                if g < 3:
                    q_proj2(g + 1, c)
                if g == 2 and c < 2:
                    wr_proj(c)
                if i == 15:
                    epilogue(0)   # acc2[0] complete after sum_o(3, 2)
            # last pair: shorten the serial tail chain by precomputing
            # acc2[1] + prt2 (valid: acc2[1]'s last other update was pair
            # (3,1)), then add the last normalized output and relu.
            preB = misc_pool.tile([128, 512], FP, tag="pre", name="preB")
            nc.vector.tensor_add(preB[:], acc2[1][:], prt2[:, 1, :])
            presum(3, 3)
            es = es_t.pop((3, 3))
            vsb = vsb_t.pop((3, 3))
            esum = esum_t.pop((3, 3))
            sumb2 = so_psum.tile([128, 512], FP, tag="so", name="sumb2")
            for kn2 in range(2):
                for par in range(2):
                    nc.tensor.matmul(sumb2[64 * par:64 * par + 64, :],
                                     ones[:, 0:A], esum[par][:, kn2, :],
                                     start=(kn2 == 0), stop=(kn2 == 1),
                                     skip_group_check=True)
            ot2 = so_psum.tile([128, 512], FP, tag="so", name="ot2")
            for kn in range(4):
                for par in range(2):
                    nc.tensor.matmul(ot2[64 * par:64 * par + 64, :],
                                     vsb[par][:, kn, :], es[par][:, kn, :],
                                     start=(kn == 0), stop=(kn == 3),
                                     skip_group_check=True)
            recipb2 = misc_pool.tile([128, 512], FP, tag="recip",
                                     name="recipB")
            otmp2 = misc_pool.tile([128, 512], FP, tag="otmp", name="otmpB")
            outsb2 = misc_pool.tile([128, 512], FP, tag="outsb", name="outsbB")
            for lo, hi in ((0, 256), (256, 512)):
                nc.vector.reciprocal_approx_fast(out=recipb2[:, lo:hi],
                                                 in_=sumb2[:, lo:hi])
                nc.vector.tensor_mul(otmp2[:, lo:hi], ot2[:, lo:hi],
                                     recipb2[:, lo:hi])
                nc.vector.tensor_add(outsb2[:, lo:hi], preB[:, lo:hi],
                                     otmp2[:, lo:hi])
                nc.vector.tensor_scalar_max(outsb2[:, lo:hi],
                                            outsb2[:, lo:hi], 0.0)
                for par in range(2):
                    nc.sync.dma_start(
                        out=out_e[:, 512 * (2 + par) + lo:512 * (2 + par) + hi],
                        in_=outsb2[64 * par:64 * par + 64, lo:hi])

    nc.finalize()
    return nc


def _stage_inputs(x, Wq, Wk, Wv, Wr):
    """Build per-core input dicts."""
    Wk_perm = np.ascontiguousarray(Wk[:, _M_OF_P].astype(np.float16))
    Wv8 = np.ascontiguousarray((Wv / 8.0).astype(np.float16))
    Wq_c = np.ascontiguousarray(Wq.astype(np.float16))
    Wr_c = np.ascontiguousarray(Wr.astype(np.float16))
    in_maps = []
    for d in range(NCORES):
        xa = np.concatenate(
            [x[4 * h + d // 2, 256 * (d % 2):256 * (d % 2) + 256, :]
             for h in range(H)], axis=0)
        xaT = np.ascontiguousarray(xa.T.astype(np.float16))
        xoT = np.ascontiguousarray(
            np.concatenate([x[4 * d + i][_M_OF_P, :].T for i in range(4)],
                           axis=1).astype(np.float16))
        in_maps.append({
            "xaT": xaT, "xoT": xoT,
            "wqk": np.concatenate([Wq_c, Wk_perm], axis=1),
            "wvr": np.concatenate([Wv8, Wr_c], axis=1),
        })
    return in_maps


_CACHED = {}


def kernel(x, Wq, Wk, Wv, Wr, _want_trace=False):
    from concourse.bass_utils import run_bass_kernel_spmd

    x = np.asarray(x, dtype=np.float32)
    in_maps = _stage_inputs(x, np.asarray(Wq, np.float32),
                            np.asarray(Wk, np.float32),
                            np.asarray(Wv, np.float32),
                            np.asarray(Wr, np.float32))

    if "nc" not in _CACHED:
        _CACHED["nc"] = build_core_graph()
    nc = _CACHED["nc"]

    res = run_bass_kernel_spmd(nc, in_maps, core_ids=list(range(NCORES)),
                               trace=_want_trace)
    _CACHED["last_result"] = res

    out = np.zeros((B, M, A), np.float32)
    for d in range(NCORES):
        o = res.results[d]["out"]  # (64, 2048) = (a, 512*i + p)
        for i in range(4):
            out[4 * d + i] = o[:, 512 * i + _P_OF_M].T
    return out


if __name__ == "__main__":
    np.random.seed(0)
    pass

